# revision 42
# baseline (speedup 1.0000x reference)
"""Trainium2 Bass kernel for AetherLoss: chamfer(recon_x, x) + beta*KL(mu, logvar).

Strategy: data-parallel over batch B=8 across 8 NeuronCores (1 point-cloud
pair + 1 latent row per core).  Per core, the 4096x4096 *negated* squared
distance matrix  -dist[n,m] = 2*x_n.y_m - |x_n|^2 - |y_m|^2  is produced by
the TensorEngine as a single K=24 matmul per tile via augmented vectors,
where every fp32 operand is split into 3 bf16 components (hi/mid/lo) so the
bf16 PE path reproduces fp32-accurate products (err ~1e-7 relative).
ScalarE stages each PSUM tile to SBUF as fp16, which lets VectorE run both
min-reductions (row max-tree over the free axis + running elementwise max
for columns) in its 2x packed mode.  The column accumulator's partition-axis
max is finished by TensorE transposes (against a DMA-built anti-diagonal
matrix; J@J=I supplies the true identity used for the |x|^2 partition-group
sums) plus free-axis reduces.  Per-core partial sums are combined on the
host (equal shard sizes -> plain means), which is the scalar "all-reduce".

The row trees are evaluated for 4 x-tiles per fused op group (3D access
patterns keep the innermost dim contiguous so the 2x mode holds), which
amortizes the fixed per-instruction DVE overhead.

Measured on trn2 (neuron-profile, min of 4): ~203 us total; engine-active:
DVE ~156 us (two passes over 16.7M distances at ~2 half-precision elements
per lane-cycle is this design's floor), ScalarE ~126 us staging, TensorE
overlapped.  Phases: ~13 us framework boot, ~23 us operand prep, ~156 us
steady-state main loop (DVE-paced), ~11 us tail + exit barrier.
"""

import numpy as np
from contextlib import ExitStack

B, D, N = 8, 3, 4096
LATENT = 256
NCORES = 8
BETA = 1.0

PT = 128            # x-tile size (matmul output partitions)
NT = N // PT        # 32 x-tiles
FC = 2048           # psum group free size (4 banks)
NG = N // FC        # 2 groups
CH = 512            # matmul moving free dim (1 psum bank)
CPG = FC // CH      # 4 chunks per group
K = 24              # augmented contraction size

_cache = {}


def _build_program():
    import concourse.bass as bass
    import concourse.tile as tile
    from concourse import bacc, mybir, bass_isa

    f32 = mybir.dt.float32
    f16 = mybir.dt.float16
    bf16 = mybir.dt.bfloat16
    i32 = mybir.dt.int32
    MAX = mybir.AluOpType.max

    nc = bacc.Bacc(trn_type="TRN2", debug=False, target_bir_lowering=False)

    # ---- per-core DRAM I/O (SPMD: same program, per-core data) ----
    xr = nc.dram_tensor("xr", [D, N], f32, kind="ExternalInput")      # recon_x[b]
    xx = nc.dram_tensor("xx", [D, N], f32, kind="ExternalInput")      # x[b]
    mu = nc.dram_tensor("mu", [LATENT], f32, kind="ExternalInput")
    lv = nc.dram_tensor("lv", [LATENT], f32, kind="ExternalInput")

    o_row = nc.dram_tensor("o_row", [128, NT], f32, kind="ExternalOutput")
    o_col = nc.dram_tensor("o_col", [128, NT], f32, kind="ExternalOutput")
    o_kl = nc.dram_tensor("o_kl", [128, 1], f32, kind="ExternalOutput")

    # internal DRAM staging for the [96,128] -> [3,4096] layout flatten
    st = {}
    for name in ("axh", "axm", "axl", "x2", "yh", "ym", "yl", "y2"):
        st[name] = nc.dram_tensor("st_" + name, [D * N], bf16)
    st_j32 = nc.dram_tensor("st_j32", [63], f32)
    st_j128 = nc.dram_tensor("st_j128", [255], f16)

    with tile.TileContext(nc) as tc, ExitStack() as ctx:
        const = ctx.enter_context(tc.tile_pool(name="const", bufs=1))
        work = ctx.enter_context(tc.tile_pool(name="work", bufs=1))
        stg = ctx.enter_context(tc.tile_pool(name="stg", bufs=2))
        psum = ctx.enter_context(tc.tile_pool(name="psum", bufs=2, space="PSUM"))

        # ================= aug operand prep =================
        # Identity matrices without iota/gpsimd: an overlapping-window DMA
        # read of [..0,1,0..] gives the anti-diagonal J (all-positive strides);
        # J @ J = I recovers the true identity on the PE.
        vec32 = work.tile([1, 63], f32, tag="vec32")
        nc.vector.memset(vec32[:], 0.0)
        nc.vector.memset(vec32[0:1, 31:32], 1.0)
        nc.sync.dma_start(st_j32.ap(), vec32[:])
        J32 = work.tile([32, 32], f32, tag="J32")
        nc.sync.dma_start(J32[:], bass.AP(st_j32, 0, [[1, 32], [1, 32]]))
        ps_i = psum.tile([32, 32], f32, tag="ptile", name="ps_i")
        nc.tensor.matmul(ps_i[:], J32[:], J32[:], start=True, stop=True)
        I32 = work.tile([32, 32], f32, tag="I32")
        nc.vector.tensor_copy(I32[:], ps_i[:])
        # indicator[k, m] = (k % 32 == m): three stacked copies of I32
        ind = work.tile([96, 32], f32, tag="ind")
        for j in range(3):
            nc.sync.dma_start(ind[32 * j:32 * (j + 1), :], I32[:])
        # fp16 anti-diagonal J128 for the column-min transpose tail
        vec128 = work.tile([1, 255], f16, tag="vec128")
        nc.vector.memset(vec128[:], 0.0)
        nc.vector.memset(vec128[0:1, 127:128], 1.0)
        nc.sync.dma_start(st_j128.ap(), vec128[:])
        J128 = const.tile([128, 128], f16, tag="J128")
        nc.sync.dma_start(J128[:], bass.AP(st_j128, 0, [[1, 128], [1, 128]]))

        # Load [3,4096] as [96,128]: partition p = d*32 + t, free n (128).
        def load96(dram):
            t = work.tile([96, 128], f32, tag=f"ld_{dram.name}", name=f"ld_{dram.name}")
            nc.gpsimd.dma_start(t[:], dram.ap().rearrange("d (t n) -> (d t) n", n=128))
            return t

        def split3(src_f32, base):
            """3-way bf16 split of an fp32 tile; returns (h, m, l) bf16 tiles."""
            p, fd = src_f32.shape
            h = work.tile([p, fd], bf16, tag=f"{base}_h", name=f"{base}_h")
            m = work.tile([p, fd], bf16, tag=f"{base}_m", name=f"{base}_m")
            l = work.tile([p, fd], bf16, tag=f"{base}_l", name=f"{base}_l")
            r = work.tile([p, fd], f32, tag=f"{base}_r", name=f"{base}_r")
            r2 = work.tile([p, fd], f32, tag=f"{base}_r2", name=f"{base}_r2")
            nc.vector.tensor_copy(h[:], src_f32[:])
            nc.vector.tensor_tensor(r[:], src_f32[:], h[:], op=mybir.AluOpType.subtract)
            nc.vector.tensor_copy(m[:], r[:])
            nc.vector.tensor_tensor(r2[:], r[:], m[:], op=mybir.AluOpType.subtract)
            nc.vector.tensor_copy(l[:], r2[:])
            return h, m, l

        def neg_sumsq(src96, base):
            """-sum_d src[d*32+t, n]^2 as a [32, 128] fp32 tile via the PE:
            out[t, n] = sum_k ind[k, t] * sq[k, n]."""
            sq = work.tile([96, 128], f32, tag=f"{base}_sq", name=f"{base}_sq")
            nc.vector.tensor_tensor(sq[:], src96[:], src96[:], op=mybir.AluOpType.mult)
            ps = psum.tile([32, 128], f32, tag="ptile", name=f"{base}_ps")
            nc.tensor.matmul(ps[:], ind[:], sq[:], start=True, stop=True)
            out = work.tile([32, 128], f32, tag=f"{base}_ss", name=f"{base}_ss")
            nc.vector.tensor_scalar_mul(out[:], ps[:], -1.0)
            return out

        # x side (stationary / lhsT) carries the +2 scale
        x96 = load96(xr)
        y96 = load96(xx)
        nx2 = neg_sumsq(x96, "x2")             # -|x|^2, [32, 128]
        ny2 = neg_sumsq(y96, "y2")             # -|y|^2, [32, 128]
        ax = work.tile([96, 128], f32, tag="ax")
        nc.vector.tensor_scalar_mul(ax[:], x96[:], 2.0)
        x2h, x2m, x2l = split3(nx2, "x2")
        y2h, y2m, y2l = split3(ny2, "y2")
        axh, axm, axl = split3(ax, "ax")
        yh, ym, yl = split3(y96, "y")

        # stage components to DRAM, flattened to the [3, 4096] row layout
        _stage_cnt = [0]

        def stage(name, t, off=0):
            # [P, 128] tile, partition p -> dram offset (off + p)*128 + n
            rows = t.shape[0]
            eng = nc.sync if _stage_cnt[0] % 2 else nc.gpsimd
            _stage_cnt[0] += 1
            eng.dma_start(
                st[name].ap()[off * 128:(off + rows) * 128].rearrange(
                    "(p n) -> p n", n=128),
                t[:])

        stage("axh", axh); stage("axm", axm); stage("axl", axl)
        stage("x2", x2h, 0); stage("x2", x2m, 32); stage("x2", x2l, 64)
        stage("yh", yh); stage("ym", ym); stage("yl", yl)
        stage("y2", y2h, 0); stage("y2", y2m, 32); stage("y2", y2l, 64)

        # assemble augmented operands [24, 4096] bf16
        augX = const.tile([K, N], bf16, tag="augX")
        augY = const.tile([K, N], bf16, tag="augY")

        def fill(dst, rows, src_name, eng):
            eng.dma_start(
                dst[rows:rows + 3, :],
                st[src_name].ap().rearrange("(d m) -> d m", d=3),
            )

        # row pairing: (axh,yh) (axh,ym) (axm,yh) (axh,yl) (axl,yh) (axm,ym)
        #              (x2trio, ones) (ones, y2trio)
        for i, (r, n_) in enumerate(
                ((0, "axh"), (3, "axh"), (6, "axm"), (9, "axh"),
                 (12, "axl"), (15, "axm"), (18, "x2"))):
            fill(augX, r, n_, nc.sync if i % 2 else nc.gpsimd)
        for i, (r, n_) in enumerate(
                ((0, "yh"), (3, "ym"), (6, "yh"), (9, "yl"),
                 (12, "yh"), (15, "ym"), (21, "y2"))):
            fill(augY, r, n_, nc.gpsimd if i % 2 else nc.sync)
        ones3 = work.tile([3, N], bf16, tag="ones3")
        nc.vector.memset(ones3[:], 1.0)
        nc.sync.dma_start(augX[21:24, :], ones3[:])
        nc.gpsimd.dma_start(augY[18:21, :], ones3[:])

        # ================= KL term (tiny; schedule early) =================
        mu2d = work.tile([128, LATENT // 128], f32, tag="mu2d")
        lv2d = work.tile([128, LATENT // 128], f32, tag="lv2d")
        nc.sync.dma_start(mu2d[:], mu.ap().rearrange("(p f) -> p f", p=128))
        nc.sync.dma_start(lv2d[:], lv.ap().rearrange("(p f) -> p f", p=128))
        klsq = work.tile([128, LATENT // 128], f32, tag="klsq")
        klex = work.tile([128, LATENT // 128], f32, tag="klex")
        klt = work.tile([128, LATENT // 128], f32, tag="klt")
        klp = work.tile([128, 1], f32, tag="klp")
        nc.vector.tensor_tensor(klsq[:], mu2d[:], mu2d[:], op=mybir.AluOpType.mult)
        nc.scalar.activation(klex[:], lv2d[:], mybir.ActivationFunctionType.Exp)
        nc.vector.tensor_tensor(klt[:], lv2d[:], klsq[:], op=mybir.AluOpType.subtract)
        nc.vector.tensor_tensor(klt[:], klt[:], klex[:], op=mybir.AluOpType.subtract)
        nc.vector.reduce_sum(klp[:], klt[:], axis=mybir.AxisListType.X)
        nc.sync.dma_start(o_kl.ap(), klp[:])

        # ================= main loop =================
        rowmax = const.tile([128, NT], f32, tag="rowmax")
        colacc = const.tile([128, N], f16, tag="colacc")

        GP = 4                       # x-tiles per tree group (amortize op overhead)
        for pg in range(NT // GP):
            # rowbuf holds GP x-tiles' staged rows: [128, (pt_local, y)]
            rowbuf = stg.tile([128, GP * N], f16, tag="rowbuf", name="rowbuf", bufs=3)
            for j in range(GP):
                pt = pg * GP + j
                for g in range(NG):
                    ptile = psum.tile([128, FC], f32, tag="ptile", name="ptile")
                    for q in range(CPG):
                        c = g * CPG + q
                        nc.tensor.matmul(
                            ptile[:, q * CH:(q + 1) * CH],
                            augX[0:K, pt * PT:(pt + 1) * PT],
                            augY[0:K, c * CH:(c + 1) * CH],
                            start=True, stop=True,
                        )
                    # ScalarE stages fp32 PSUM -> fp16 SBUF
                    nc.scalar.copy(
                        rowbuf[:, j * N + g * FC:j * N + (g + 1) * FC], ptile[:])
                # column accumulator update per x-tile, then this tile's
                # tree level 1 right away (starts the tree before the whole
                # group is staged -> smoother DVE pipeline)
                rb = rowbuf[:, j * N:(j + 1) * N]
                if pt == 0:
                    nc.vector.tensor_copy(colacc[:], rb)
                else:
                    nc.vector.tensor_tensor(colacc[:], colacc[:], rb, op=MAX)
                if j == 0:
                    t1 = stg.tile([128, GP * 2048], f16, tag="t1", name="t1")
                nc.vector.tensor_tensor(
                    t1[:, j * 2048:(j + 1) * 2048],
                    rowbuf[:, j * N:j * N + 2048],
                    rowbuf[:, j * N + 2048:(j + 1) * N], op=MAX)
            # fused fp16 max-tree over the remaining levels for all GP
            # x-tiles (3D APs keep the innermost dim step-1 so 2x holds)
            t1v = t1[:].rearrange("p (j n) -> p j n", j=GP)
            t2 = stg.tile([128, GP * 1024], f16, tag="t2", name="t2")
            t2v = t2[:].rearrange("p (j n) -> p j n", j=GP)
            nc.vector.tensor_tensor(t2v, t1v[:, :, 0:1024], t1v[:, :, 1024:2048], op=MAX)
            t3 = stg.tile([128, GP * 512], f16, tag="t3", name="t3")
            t3v = t3[:].rearrange("p (j n) -> p j n", j=GP)
            nc.vector.tensor_tensor(t3v, t2v[:, :, 0:512], t2v[:, :, 512:1024], op=MAX)
            t4 = stg.tile([128, GP * 256], f16, tag="t4", name="t4")
            t4v = t4[:].rearrange("p (j n) -> p j n", j=GP)
            nc.vector.tensor_tensor(t4v, t3v[:, :, 0:256], t3v[:, :, 256:512], op=MAX)
            t5 = stg.tile([128, GP * 128], f16, tag="t5", name="t5")
            t5v = t5[:].rearrange("p (j n) -> p j n", j=GP)
            nc.vector.tensor_tensor(t5v, t4v[:, :, 0:128], t4v[:, :, 128:256], op=MAX)
            t6 = stg.tile([128, GP * 64], f16, tag="t6", name="t6")
            t6v = t6[:].rearrange("p (j n) -> p j n", j=GP)
            nc.vector.tensor_tensor(t6v, t5v[:, :, 0:64], t5v[:, :, 64:128], op=MAX)
            nc.vector.tensor_reduce(rowmax[:, pg * GP:(pg + 1) * GP], t6v,
                                    axis=mybir.AxisListType.X, op=MAX)

        # ================= tails =================
        nc.sync.dma_start(o_row.ap(), rowmax[:])
        # column mins: PE-transpose each [128,128] block of colacc (with the
        # anti-diagonal J128 -> free axis is x reversed, irrelevant for max),
        # then free-axis max-reduce the transposed blocks.
        coltail = work.tile([128, NT], f32, tag="coltail")
        for h in range(4):  # 8 blocks per psum tile
            tp = psum.tile([128, 1024], f16, tag="ptile", name="tp")
            for b in range(8):
                blk = h * 8 + b
                nc.tensor.transpose(
                    tp[:, b * 128:(b + 1) * 128],
                    colacc[:, blk * 128:(blk + 1) * 128],
                    J128[:])
            nc.vector.tensor_reduce(
                coltail[:, h * 8:(h + 1) * 8],
                tp[:].rearrange("p (b n) -> p b n", n=128),
                axis=mybir.AxisListType.X, op=MAX)
        # coltail[p, blk] corresponds to y = blk*128 + p; host only sums it,
        # so store it in its natural [128, 32] layout (fast contiguous DMA)
        nc.sync.dma_start(o_col.ap(), coltail[:])

    nc.compile()
    return nc


def _get_nc():
    if "nc" not in _cache:
        _cache["nc"] = _build_program()
    return _cache["nc"]


def _register_ntff_hook():
    """This image's antenv lacks axon_hooks; register the NTFF profile hook
    ourselves so run_bass_kernel_spmd(trace=True) can neuron-profile."""
    import sys, types
    if "antenv.axon_hooks" in sys.modules:
        return
    try:
        from trn_agent_boot.trn_boot import _ntff_profile_via_ctypes
        hook = _ntff_profile_via_ctypes("/opt/axon/libaxon_pjrt.so")
        mod = types.ModuleType("antenv.axon_hooks")
        mod.get_axon_ntff_profile_hook = lambda: hook
        mod.set_axon_ntff_profile_hook = lambda h: None
        sys.modules["antenv.axon_hooks"] = mod
        from concourse import bass_utils
        bass_utils.upload_artifacts = lambda tmpdir: tmpdir
    except Exception:
        pass


def _run(in_maps, trace=False):
    from concourse.bass_utils import run_bass_kernel_spmd
    if trace:
        _register_ntff_hook()
    nc = _get_nc()
    return run_bass_kernel_spmd(nc, in_maps, list(range(NCORES)), trace=trace)


def _combine(results):
    minx_sum = 0.0
    miny_sum = 0.0
    kl_sum = 0.0
    for r in results:
        minx_sum += -(r["o_row"].astype(np.float64).sum())
        miny_sum += -(r["o_col"].astype(np.float64).sum())
        kl_sum += r["o_kl"].astype(np.float64).sum()
    recon = minx_sum / (NCORES * N) + miny_sum / (NCORES * N)
    kld = -0.5 * (B * LATENT * 1.0 + kl_sum) / B
    total = recon + BETA * kld
    return (np.float32(total), np.float32(recon), np.float32(kld))


def kernel(recon_x, x, mu, logvar, _trace=False):
    recon_x = np.ascontiguousarray(recon_x, dtype=np.float32)
    x = np.ascontiguousarray(x, dtype=np.float32)
    mu = np.ascontiguousarray(mu, dtype=np.float32)
    logvar = np.ascontiguousarray(logvar, dtype=np.float32)
    in_maps = [
        {"xr": recon_x[c], "xx": x[c], "mu": mu[c], "lv": logvar[c]}
        for c in range(NCORES)
    ]
    res = _run(in_maps, trace=_trace)
    out = _combine(res.results)
    if _trace:
        return out, res
    return out


# revision 43
# speedup vs baseline: 1.0028x; 1.0028x over previous
"""Trainium2 Bass kernel for AetherLoss: chamfer(recon_x, x) + beta*KL(mu, logvar).

Strategy: data-parallel over batch B=8 across 8 NeuronCores (1 point-cloud
pair + 1 latent row per core).  Per core, the 4096x4096 *negated* squared
distance matrix  -dist[n,m] = 2*x_n.y_m - |x_n|^2 - |y_m|^2  is produced by
the TensorEngine as a single K=24 matmul per tile via augmented vectors,
where every fp32 operand is split into 3 bf16 components (hi/mid/lo) so the
bf16 PE path reproduces fp32-accurate products (err ~1e-7 relative).
ScalarE stages each PSUM tile to SBUF as fp16, which lets VectorE run both
min-reductions (row max-tree over the free axis + running elementwise max
for columns) in its 2x packed mode.  The column accumulator's partition-axis
max is finished by TensorE transposes (against a DMA-built anti-diagonal
matrix; J@J=I supplies the true identity used for the |x|^2 partition-group
sums) plus free-axis reduces.  Per-core partial sums are combined on the
host (equal shard sizes -> plain means), which is the scalar "all-reduce".

The row trees are evaluated for 4 x-tiles per fused op group (3D access
patterns keep the innermost dim contiguous so the 2x mode holds), which
amortizes the fixed per-instruction DVE overhead.

Measured on trn2 (neuron-profile, min of 4): ~203 us total; engine-active:
DVE ~156 us (two passes over 16.7M distances at ~2 half-precision elements
per lane-cycle is this design's floor), ScalarE ~126 us staging, TensorE
overlapped.  Phases: ~13 us framework boot, ~23 us operand prep, ~156 us
steady-state main loop (DVE-paced), ~11 us tail + exit barrier.
"""

import numpy as np
from contextlib import ExitStack

B, D, N = 8, 3, 4096
LATENT = 256
NCORES = 8
BETA = 1.0

PT = 128            # x-tile size (matmul output partitions)
NT = N // PT        # 32 x-tiles
FC = 2048           # psum group free size (4 banks)
NG = N // FC        # 2 groups
CH = 512            # matmul moving free dim (1 psum bank)
CPG = FC // CH      # 4 chunks per group
K = 24              # augmented contraction size

_cache = {}


def _build_program():
    import concourse.bass as bass
    import concourse.tile as tile
    from concourse import bacc, mybir, bass_isa

    f32 = mybir.dt.float32
    f16 = mybir.dt.float16
    bf16 = mybir.dt.bfloat16
    i32 = mybir.dt.int32
    MAX = mybir.AluOpType.max

    nc = bacc.Bacc(trn_type="TRN2", debug=False, target_bir_lowering=False)

    # ---- per-core DRAM I/O (SPMD: same program, per-core data) ----
    xr = nc.dram_tensor("xr", [D, N], f32, kind="ExternalInput")      # recon_x[b]
    xx = nc.dram_tensor("xx", [D, N], f32, kind="ExternalInput")      # x[b]
    mu = nc.dram_tensor("mu", [LATENT], f32, kind="ExternalInput")
    lv = nc.dram_tensor("lv", [LATENT], f32, kind="ExternalInput")

    o_row = nc.dram_tensor("o_row", [128, NT], f32, kind="ExternalOutput")
    o_col = nc.dram_tensor("o_col", [128, NT], f32, kind="ExternalOutput")
    o_kl = nc.dram_tensor("o_kl", [128, 1], f32, kind="ExternalOutput")

    # internal DRAM staging for the [96,128] -> [3,4096] layout flatten
    st = {}
    for name in ("axh", "axm", "axl", "x2", "yh", "ym", "yl", "y2"):
        st[name] = nc.dram_tensor("st_" + name, [D * N], bf16)
    st_j32 = nc.dram_tensor("st_j32", [63], f32)
    st_j128 = nc.dram_tensor("st_j128", [255], f16)

    with tile.TileContext(nc) as tc, ExitStack() as ctx:
        const = ctx.enter_context(tc.tile_pool(name="const", bufs=1))
        work = ctx.enter_context(tc.tile_pool(name="work", bufs=1))
        stg = ctx.enter_context(tc.tile_pool(name="stg", bufs=2))
        psum = ctx.enter_context(tc.tile_pool(name="psum", bufs=2, space="PSUM"))

        # ================= aug operand prep =================
        # Identity matrices without iota/gpsimd: an overlapping-window DMA
        # read of [..0,1,0..] gives the anti-diagonal J (all-positive strides);
        # J @ J = I recovers the true identity on the PE.
        vec32 = work.tile([1, 63], f32, tag="vec32")
        nc.vector.memset(vec32[:], 0.0)
        nc.vector.memset(vec32[0:1, 31:32], 1.0)
        nc.sync.dma_start(st_j32.ap(), vec32[:])
        J32 = work.tile([32, 32], f32, tag="J32")
        nc.sync.dma_start(J32[:], bass.AP(st_j32, 0, [[1, 32], [1, 32]]))
        ps_i = psum.tile([32, 32], f32, tag="ptile", name="ps_i")
        nc.tensor.matmul(ps_i[:], J32[:], J32[:], start=True, stop=True)
        I32 = work.tile([32, 32], f32, tag="I32")
        nc.vector.tensor_copy(I32[:], ps_i[:])
        # indicator[k, m] = (k % 32 == m): three stacked copies of I32
        ind = work.tile([96, 32], f32, tag="ind")
        for j in range(3):
            nc.sync.dma_start(ind[32 * j:32 * (j + 1), :], I32[:])
        # fp16 anti-diagonal J128 for the column-min transpose tail
        vec128 = work.tile([1, 255], f16, tag="vec128")
        nc.vector.memset(vec128[:], 0.0)
        nc.vector.memset(vec128[0:1, 127:128], 1.0)
        nc.sync.dma_start(st_j128.ap(), vec128[:])
        J128 = const.tile([128, 128], f16, tag="J128")
        nc.sync.dma_start(J128[:], bass.AP(st_j128, 0, [[1, 128], [1, 128]]))

        # Load [3,4096] as [96,128]: partition p = d*32 + t, free n (128).
        def load96(dram):
            t = work.tile([96, 128], f32, tag=f"ld_{dram.name}", name=f"ld_{dram.name}")
            nc.gpsimd.dma_start(t[:], dram.ap().rearrange("d (t n) -> (d t) n", n=128))
            return t

        def split3(src_f32, base):
            """3-way bf16 split of an fp32 tile; returns (h, m, l) bf16 tiles."""
            p, fd = src_f32.shape
            h = work.tile([p, fd], bf16, tag=f"{base}_h", name=f"{base}_h")
            m = work.tile([p, fd], bf16, tag=f"{base}_m", name=f"{base}_m")
            l = work.tile([p, fd], bf16, tag=f"{base}_l", name=f"{base}_l")
            r = work.tile([p, fd], f32, tag=f"{base}_r", name=f"{base}_r")
            r2 = work.tile([p, fd], f32, tag=f"{base}_r2", name=f"{base}_r2")
            nc.vector.tensor_copy(h[:], src_f32[:])
            nc.vector.tensor_tensor(r[:], src_f32[:], h[:], op=mybir.AluOpType.subtract)
            nc.vector.tensor_copy(m[:], r[:])
            nc.vector.tensor_tensor(r2[:], r[:], m[:], op=mybir.AluOpType.subtract)
            nc.vector.tensor_copy(l[:], r2[:])
            return h, m, l

        def neg_sumsq(src96, base):
            """-sum_d src[d*32+t, n]^2 as a [32, 128] fp32 tile via the PE:
            out[t, n] = sum_k ind[k, t] * sq[k, n]."""
            sq = work.tile([96, 128], f32, tag=f"{base}_sq", name=f"{base}_sq")
            nc.vector.tensor_tensor(sq[:], src96[:], src96[:], op=mybir.AluOpType.mult)
            ps = psum.tile([32, 128], f32, tag="ptile", name=f"{base}_ps")
            nc.tensor.matmul(ps[:], ind[:], sq[:], start=True, stop=True)
            out = work.tile([32, 128], f32, tag=f"{base}_ss", name=f"{base}_ss")
            nc.vector.tensor_scalar_mul(out[:], ps[:], -1.0)
            return out

        # x side (stationary / lhsT) carries the +2 scale
        x96 = load96(xr)
        y96 = load96(xx)
        nx2 = neg_sumsq(x96, "x2")             # -|x|^2, [32, 128]
        ny2 = neg_sumsq(y96, "y2")             # -|y|^2, [32, 128]
        ax = work.tile([96, 128], f32, tag="ax")
        nc.vector.tensor_scalar_mul(ax[:], x96[:], 2.0)
        x2h, x2m, x2l = split3(nx2, "x2")
        y2h, y2m, y2l = split3(ny2, "y2")
        axh, axm, axl = split3(ax, "ax")
        yh, ym, yl = split3(y96, "y")

        # stage components to DRAM, flattened to the [3, 4096] row layout
        _stage_cnt = [0]

        def stage(name, t, off=0):
            # [P, 128] tile, partition p -> dram offset (off + p)*128 + n
            rows = t.shape[0]
            eng = nc.sync if _stage_cnt[0] % 2 else nc.gpsimd
            _stage_cnt[0] += 1
            eng.dma_start(
                st[name].ap()[off * 128:(off + rows) * 128].rearrange(
                    "(p n) -> p n", n=128),
                t[:])

        stage("axh", axh); stage("axm", axm); stage("axl", axl)
        stage("x2", x2h, 0); stage("x2", x2m, 32); stage("x2", x2l, 64)
        stage("yh", yh); stage("ym", ym); stage("yl", yl)
        stage("y2", y2h, 0); stage("y2", y2m, 32); stage("y2", y2l, 64)

        # assemble augmented operands [24, 4096] bf16
        augX = const.tile([K, N], bf16, tag="augX")
        augY = const.tile([K, N], bf16, tag="augY")

        def fill(dst, rows, src_name, eng):
            eng.dma_start(
                dst[rows:rows + 3, :],
                st[src_name].ap().rearrange("(d m) -> d m", d=3),
            )

        # row pairing: (axh,yh) (axh,ym) (axm,yh) (axh,yl) (axl,yh) (axm,ym)
        #              (x2trio, ones) (ones, y2trio)
        for i, (r, n_) in enumerate(
                ((0, "axh"), (3, "axh"), (6, "axm"), (9, "axh"),
                 (12, "axl"), (15, "axm"), (18, "x2"))):
            fill(augX, r, n_, nc.sync if i % 2 else nc.gpsimd)
        for i, (r, n_) in enumerate(
                ((0, "yh"), (3, "ym"), (6, "yh"), (9, "yl"),
                 (12, "yh"), (15, "ym"), (21, "y2"))):
            fill(augY, r, n_, nc.gpsimd if i % 2 else nc.sync)
        ones3 = work.tile([3, N], bf16, tag="ones3")
        nc.vector.memset(ones3[:], 1.0)
        nc.sync.dma_start(augX[21:24, :], ones3[:])
        nc.gpsimd.dma_start(augY[18:21, :], ones3[:])

        # ================= KL term (tiny; schedule early) =================
        mu2d = work.tile([128, LATENT // 128], f32, tag="mu2d")
        lv2d = work.tile([128, LATENT // 128], f32, tag="lv2d")
        nc.sync.dma_start(mu2d[:], mu.ap().rearrange("(p f) -> p f", p=128))
        nc.sync.dma_start(lv2d[:], lv.ap().rearrange("(p f) -> p f", p=128))
        klsq = work.tile([128, LATENT // 128], f32, tag="klsq")
        klex = work.tile([128, LATENT // 128], f32, tag="klex")
        klt = work.tile([128, LATENT // 128], f32, tag="klt")
        klp = work.tile([128, 1], f32, tag="klp")
        nc.vector.tensor_tensor(klsq[:], mu2d[:], mu2d[:], op=mybir.AluOpType.mult)
        nc.scalar.activation(klex[:], lv2d[:], mybir.ActivationFunctionType.Exp)
        nc.vector.tensor_tensor(klt[:], lv2d[:], klsq[:], op=mybir.AluOpType.subtract)
        nc.vector.tensor_tensor(klt[:], klt[:], klex[:], op=mybir.AluOpType.subtract)
        nc.vector.reduce_sum(klp[:], klt[:], axis=mybir.AxisListType.X)
        nc.sync.dma_start(o_kl.ap(), klp[:])

        # ================= main loop =================
        rowmax = const.tile([128, NT], f32, tag="rowmax")
        colacc = const.tile([128, N], f16, tag="colacc")

        GP = 4                       # x-tiles per tree group (amortize op overhead)
        for pg in range(NT // GP):
            # rowbuf holds GP x-tiles' staged rows: [128, (pt_local, y)]
            rowbuf = stg.tile([128, GP * N], f16, tag="rowbuf", name="rowbuf")
            for j in range(GP):
                pt = pg * GP + j
                for g in range(NG):
                    ptile = psum.tile([128, FC], f32, tag="ptile", name="ptile")
                    for q in range(CPG):
                        c = g * CPG + q
                        nc.tensor.matmul(
                            ptile[:, q * CH:(q + 1) * CH],
                            augX[0:K, pt * PT:(pt + 1) * PT],
                            augY[0:K, c * CH:(c + 1) * CH],
                            start=True, stop=True,
                        )
                    # ScalarE stages fp32 PSUM -> fp16 SBUF
                    nc.scalar.copy(
                        rowbuf[:, j * N + g * FC:j * N + (g + 1) * FC], ptile[:])
                # column accumulator update per x-tile, then this tile's
                # tree level 1 right away (starts the tree before the whole
                # group is staged -> smoother DVE pipeline)
                rb = rowbuf[:, j * N:(j + 1) * N]
                if pt == 0:
                    nc.vector.tensor_copy(colacc[:], rb)
                else:
                    nc.vector.tensor_tensor(colacc[:], colacc[:], rb, op=MAX)
                if j == 0:
                    t1 = stg.tile([128, GP * 2048], f16, tag="t1", name="t1")
                nc.vector.tensor_tensor(
                    t1[:, j * 2048:(j + 1) * 2048],
                    rowbuf[:, j * N:j * N + 2048],
                    rowbuf[:, j * N + 2048:(j + 1) * N], op=MAX)
            # fused fp16 max-tree over the remaining levels for all GP
            # x-tiles (3D APs keep the innermost dim step-1 so 2x holds)
            t1v = t1[:].rearrange("p (j n) -> p j n", j=GP)
            t2 = stg.tile([128, GP * 1024], f16, tag="t2", name="t2")
            t2v = t2[:].rearrange("p (j n) -> p j n", j=GP)
            nc.vector.tensor_tensor(t2v, t1v[:, :, 0:1024], t1v[:, :, 1024:2048], op=MAX)
            t3 = stg.tile([128, GP * 512], f16, tag="t3", name="t3")
            t3v = t3[:].rearrange("p (j n) -> p j n", j=GP)
            nc.vector.tensor_tensor(t3v, t2v[:, :, 0:512], t2v[:, :, 512:1024], op=MAX)
            t4 = stg.tile([128, GP * 256], f16, tag="t4", name="t4")
            t4v = t4[:].rearrange("p (j n) -> p j n", j=GP)
            nc.vector.tensor_tensor(t4v, t3v[:, :, 0:256], t3v[:, :, 256:512], op=MAX)
            t5 = stg.tile([128, GP * 128], f16, tag="t5", name="t5")
            t5v = t5[:].rearrange("p (j n) -> p j n", j=GP)
            nc.vector.tensor_tensor(t5v, t4v[:, :, 0:128], t4v[:, :, 128:256], op=MAX)
            t6 = stg.tile([128, GP * 64], f16, tag="t6", name="t6")
            t6v = t6[:].rearrange("p (j n) -> p j n", j=GP)
            nc.vector.tensor_tensor(t6v, t5v[:, :, 0:64], t5v[:, :, 64:128], op=MAX)
            nc.vector.tensor_reduce(rowmax[:, pg * GP:(pg + 1) * GP], t6v,
                                    axis=mybir.AxisListType.X, op=MAX)

        # ================= tails =================
        nc.sync.dma_start(o_row.ap(), rowmax[:])
        # column mins: PE-transpose each [128,128] block of colacc (with the
        # anti-diagonal J128 -> free axis is x reversed, irrelevant for max),
        # then free-axis max-reduce the transposed blocks.
        coltail = work.tile([128, NT], f32, tag="coltail")
        for h in range(4):  # 8 blocks per psum tile
            tp = psum.tile([128, 1024], f16, tag="ptile", name="tp")
            for b in range(8):
                blk = h * 8 + b
                nc.tensor.transpose(
                    tp[:, b * 128:(b + 1) * 128],
                    colacc[:, blk * 128:(blk + 1) * 128],
                    J128[:])
            nc.vector.tensor_reduce(
                coltail[:, h * 8:(h + 1) * 8],
                tp[:].rearrange("p (b n) -> p b n", n=128),
                axis=mybir.AxisListType.X, op=MAX)
        # coltail[p, blk] corresponds to y = blk*128 + p; host only sums it,
        # so store it in its natural [128, 32] layout (fast contiguous DMA)
        nc.sync.dma_start(o_col.ap(), coltail[:])

    nc.compile()
    return nc


def _get_nc():
    if "nc" not in _cache:
        _cache["nc"] = _build_program()
    return _cache["nc"]


def _register_ntff_hook():
    """This image's antenv lacks axon_hooks; register the NTFF profile hook
    ourselves so run_bass_kernel_spmd(trace=True) can neuron-profile."""
    import sys, types
    if "antenv.axon_hooks" in sys.modules:
        return
    try:
        from trn_agent_boot.trn_boot import _ntff_profile_via_ctypes
        hook = _ntff_profile_via_ctypes("/opt/axon/libaxon_pjrt.so")
        mod = types.ModuleType("antenv.axon_hooks")
        mod.get_axon_ntff_profile_hook = lambda: hook
        mod.set_axon_ntff_profile_hook = lambda h: None
        sys.modules["antenv.axon_hooks"] = mod
        from concourse import bass_utils
        bass_utils.upload_artifacts = lambda tmpdir: tmpdir
    except Exception:
        pass


def _run(in_maps, trace=False):
    from concourse.bass_utils import run_bass_kernel_spmd
    if trace:
        _register_ntff_hook()
    nc = _get_nc()
    return run_bass_kernel_spmd(nc, in_maps, list(range(NCORES)), trace=trace)


def _combine(results):
    minx_sum = 0.0
    miny_sum = 0.0
    kl_sum = 0.0
    for r in results:
        minx_sum += -(r["o_row"].astype(np.float64).sum())
        miny_sum += -(r["o_col"].astype(np.float64).sum())
        kl_sum += r["o_kl"].astype(np.float64).sum()
    recon = minx_sum / (NCORES * N) + miny_sum / (NCORES * N)
    kld = -0.5 * (B * LATENT * 1.0 + kl_sum) / B
    total = recon + BETA * kld
    return (np.float32(total), np.float32(recon), np.float32(kld))


def kernel(recon_x, x, mu, logvar, _trace=False):
    recon_x = np.ascontiguousarray(recon_x, dtype=np.float32)
    x = np.ascontiguousarray(x, dtype=np.float32)
    mu = np.ascontiguousarray(mu, dtype=np.float32)
    logvar = np.ascontiguousarray(logvar, dtype=np.float32)
    in_maps = [
        {"xr": recon_x[c], "xx": x[c], "mu": mu[c], "lv": logvar[c]}
        for c in range(NCORES)
    ]
    res = _run(in_maps, trace=_trace)
    out = _combine(res.results)
    if _trace:
        return out, res
    return out
